# revision 32
# baseline (speedup 1.0000x reference)
"""Trainium2 Bass kernel for nn_BotUpSaliency (B=2, H=W=512, K=12, 16 steps).

Math
----
The reference integrates, for 16 Euler steps (EPS=0.01):

    y'  = y + EPS*(-y + gx + conv(gx,W) + 1)
    x'  = x + EPS*(J0*gx + conv(gx,J) + inputs + i_norm - x - gy - gy@psi)
    gx  = clip(x - 1, 0, 1),  gy piecewise-linear,  out = mean_t gx_t, max over K

with x0 = 0.01, y0 = 1.  While gx == 0 (everywhere), the system collapses
exactly:
  * y stays exactly 1.0  (y + 0.01*(-1 + 0 + 0 + 1) == y), so gy == 0.21.
  * i_norm == 0.85 (conv of the all-zero s), conv(gx,*) == 0.
  * x_t = a_t * inputs + b_t elementwise, with scalar recurrences
        a_{t+1} = (1-EPS) a_t + EPS,           a_0 = 0
        b_{t+1} = (1-EPS) b_t + EPS*(0.85 - gy - colsum(psi)*gy),  b_0 = 0.01
  * gx_t = clip(a_t*inputs + b_t - 1, 0, 1) stays identically 0 as long as
        max_t (a_t * inputs.max() + b_t) < 1
    which requires inputs.max() >= ~6.66; the model's input domain is [0,1).

Hence out = (1/16) * sum_t clip(a_t*inputs + b_t - 1, 0, 1), and because each
term is nondecreasing in the input value, max over channels commutes with the
whole expression: it is evaluated at m = max_k inputs.

The device kernel computes exactly that:  m = channel-max of the input slab
(reads all input bytes - the memory-bound part), then accumulates the 16
affine-clip terms and writes m's row block of the saliency map.

A host-side guard verifies the collapse precondition (with wide margin) from
the actual inputs/psi and otherwise falls back to a full jax implementation
of the reference on CPU.

Sharding: pure data parallelism, 8 cores x 128 rows of the flattened
(2*512, 512, 12) input.
"""

import numpy as np

K = 12
STEPS = 16
EPS = 0.01
TX = 1.0
G1 = 0.21
J0 = 0.8
B, H, WD = 2, 512, 512
N_CORES = 8
ROWS = B * H                  # 1024 flattened rows
RPC = ROWS // N_CORES         # 128 rows per core == SBUF partitions
ROWW = WD * K                  # 6144 floats per row
# input is staged channel-major (host transpose): 12 planes of [rows, 512];
# DMA chunks of 2 planes pipeline across the two HWDGE rings
CHUNK_PLANES = (2, 2, 2, 2, 1, 1, 1, 1)
assert sum(CHUNK_PLANES) == K

_CACHE = {}


def _coeffs(colsum):
    """Scalar affine recurrence coefficients while gx == 0 (float64)."""
    gy = G1 * 1.0             # y stays exactly 1.0
    drive = 0.85 - gy - colsum * gy
    a, b = 0.0, 0.01
    A, Bc = [], []
    for _ in range(STEPS):
        a = (1.0 - EPS) * a + EPS
        b = (1.0 - EPS) * b + EPS * drive
        A.append(a)
        Bc.append(b)
    return np.array(A), np.array(Bc)


def _build_program(A, Bc):
    import concourse.bacc as bacc
    import concourse.mybir as mybir
    from concourse.tile import TileContext

    f32 = mybir.dt.float32
    bf16 = mybir.dt.bfloat16
    relu = mybir.ActivationFunctionType.Relu

    nc = bacc.Bacc("TRN2", target_bir_lowering=False, debug=False)
    x = nc.dram_tensor("x", [RPC, ROWW], bf16, kind="ExternalInput")
    out = nc.dram_tensor("out", [RPC, WD], f32, kind="ExternalOutput")

    with TileContext(nc) as tc:
        with (
            tc.tile_pool(name="inp", bufs=8) as inpool,
            tc.tile_pool(name="zs", bufs=8) as zpool,
            tc.tile_pool(name="one", bufs=1) as spool,
        ):
            # per-step activation biases (b_t - 1)/16 as [128,1] scalars; the
            # 1/16 out-scale is folded into scale/bias/min so acc IS the output
            btab = spool.tile([RPC, STEPS], f32, tag="btab")
            for s in range(STEPS):
                nc.gpsimd.memset(btab[:, s:s + 1], float((Bc[s] - 1.0) / STEPS))
            # warm the ACT Relu table during the DMA window (1.3us table load)
            zw = spool.tile([RPC, 1], f32, tag="zw")
            nc.scalar.activation(out=zw[:], in_=btab[:, 0:1], func=relu)
            # m = per-pixel channel max: pairwise bf16 TT maxes over channel
            # planes (334ns each in 2x mode vs ~1.5us strided reduce); chunk
            # DMAs alternate between the two HWDGE rings (SP + ACT)
            m = spool.tile([RPC, WD], bf16, tag="m")
            running = None
            col = 0
            for c, npl in enumerate(CHUNK_PLANES):
                t = inpool.tile([RPC, npl * WD], bf16, tag=f"in{npl}", name="t")
                dma_eng = nc.sync if c % 2 == 0 else nc.scalar
                dma_eng.dma_start(out=t[:], in_=x[:, col:col + npl * WD])
                col += npl * WD
                last = c == len(CHUNK_PLANES) - 1
                if npl == 2:
                    p = zpool.tile([RPC, WD], bf16, tag="pp", name="pp")
                    nc.vector.tensor_tensor(
                        out=p[:], in0=t[:, :WD], in1=t[:, WD:],
                        op=mybir.AluOpType.max)
                else:
                    p = t
                if running is None:
                    running = p
                else:
                    nxt = m if last else zpool.tile(
                        [RPC, WD], bf16, tag="rm", name="rm")
                    nc.vector.tensor_tensor(
                        out=nxt[:], in0=running[:], in1=p[:, :WD],
                        op=mybir.AluOpType.max)
                    running = nxt
            # acc = sum_t clip(a_t*m + b_t - 1, 0, 1); ACT does the affine+relu,
            # DVE runs the fused (min 1) + acc chain (STT is DVE-only; Pool's
            # tensor_scalar ucode is ~15x slower and contends on the SBUF port).
            accA = spool.tile([RPC, WD], f32, tag="accA", name="accA")
            accB = spool.tile([RPC, WD], f32, tag="accB", name="accB")
            inv = 1.0 / STEPS
            for s in range(STEPS):
                z = zpool.tile([RPC, WD], f32, tag="z")
                nc.scalar.activation(
                    out=z[:], in_=m[:], func=relu,
                    bias=btab[:, s:s + 1], scale=float(A[s] / STEPS),
                )
                # two interleaved chains dodge the same-engine RAW penalty
                acc = accA if s % 2 == 0 else accB
                if s < 2:
                    nc.vector.tensor_scalar_min(out=acc[:], in0=z[:], scalar1=inv)
                else:
                    nc.vector.scalar_tensor_tensor(
                        out=acc[:], in0=z[:], scalar=inv, in1=acc[:],
                        op0=mybir.AluOpType.min, op1=mybir.AluOpType.add,
                    )
            nc.vector.tensor_add(out=accA[:], in0=accA[:], in1=accB[:])
            nc.sync.dma_start(out=out[:], in_=accA[:])

    nc.compile()
    return nc


def _get_program(A, Bc):
    key = (tuple(np.round(A, 12)), tuple(np.round(Bc, 12)))
    if key not in _CACHE:
        _CACHE[key] = _build_program(A, Bc)
    return _CACHE[key]


def _run_on_device(inputs_np, A, Bc, trace=False):
    from concourse.bass_utils import run_bass_kernel_spmd

    nc = _get_program(A, Bc)
    import ml_dtypes
    flat = np.ascontiguousarray(
        inputs_np.reshape(ROWS, WD, K).transpose(0, 2, 1)
    ).astype(ml_dtypes.bfloat16).reshape(ROWS, ROWW)
    in_maps = [
        {"x": np.ascontiguousarray(flat[i * RPC:(i + 1) * RPC])}
        for i in range(N_CORES)
    ]
    res = run_bass_kernel_spmd(nc, in_maps, list(range(N_CORES)), trace=trace)
    out = np.concatenate([res.results[i]["out"] for i in range(N_CORES)], axis=0)
    return out.reshape(B, H, WD).astype(np.float32), res


def _reference_fallback(inputs, Wk, Jk, psi):
    """Full reference math in jax on CPU (only for out-of-domain inputs)."""
    import jax
    import jax.numpy as jnp

    cpu = jax.devices("cpu")[0]
    with jax.default_device(cpu):
        inputs = jnp.asarray(np.asarray(inputs), jnp.float32)
        Wk = jnp.asarray(np.asarray(Wk), jnp.float32)
        Jk = jnp.asarray(np.asarray(Jk), jnp.float32)
        psi = jnp.asarray(np.asarray(psi), jnp.float32)
        PAD = 7

        def _conv(xx, kk, padding):
            return jax.lax.conv_general_dilated(
                xx, kk, (1, 1), padding,
                dimension_numbers=("NHWC", "HWIO", "NHWC"))

        def _gx(xx):
            return jnp.clip(xx - TX, 0.0, 1.0)

        def _gy(yy):
            yc = jnp.maximum(yy, 0.0)
            return jnp.where(yc <= 1.2, G1 * yc, G1 * 1.2 + 2.5 * (yc - 1.2))

        psi_mat = psi[0, 0]
        box = jnp.ones((5, 5, 1, 1), inputs.dtype)
        x = jnp.full_like(inputs, 0.01)
        y = jnp.ones_like(inputs)
        gx = _gx(x)
        gy = _gy(y)
        out = jnp.zeros_like(inputs)
        for _ in range(STEPS):
            s = jnp.sum(gx, axis=3, keepdims=True)
            i_norm = 0.85 - 2.0 * (_conv(s, box, "SAME") / 25.0) ** 2
            gx_p = jnp.pad(gx, ((0, 0), (PAD, PAD), (PAD, PAD), (0, 0)),
                           mode="symmetric")
            inhib = _conv(gx_p, Wk, "VALID")
            excit = _conv(gx_p, Jk, "VALID")
            inhibs_psi = jnp.einsum("bhwi,io->bhwo", gy, psi_mat)
            y_new = y + EPS * (-y + gx + inhib + 1.0)
            x_inhib = x + gy + inhibs_psi
            x_excit = J0 * gx + excit + inputs + i_norm
            x_new = x + EPS * (x_excit - x_inhib)
            gx = _gx(x_new)
            gy = _gy(y_new)
            x, y = x_new, y_new
            out = out + gx
        out = out / STEPS
        return np.asarray(jnp.max(out, axis=3))


def kernel(inputs, W=None, J=None, psi=None, **_ignored):
    inputs_np = np.asarray(inputs, dtype=np.float32)
    assert inputs_np.shape == (B, H, WD, K), inputs_np.shape

    # Guard: the gx==0 collapse must hold for these inputs/psi.
    ok = True
    colsum = 3.0
    if psi is not None:
        cs = np.asarray(psi, dtype=np.float64)[0, 0].sum(axis=0)
        if np.max(np.abs(cs - cs[0])) < 1e-9:
            colsum = float(cs[0])
        else:
            ok = False
    if ok:
        A, Bc = _coeffs(colsum)
        # 1.004 factor covers bf16 round-up of the staged inputs (<= 2^-8 rel)
        mx = float(inputs_np.max()) * 1.004
        if np.max(A * mx + Bc) >= 0.98:
            ok = False
    if not ok:
        return _reference_fallback(inputs, W, J, psi).astype(np.float32)

    out, _ = _run_on_device(inputs_np, A, Bc)
    return out


if __name__ == "__main__":
    rng = np.random.default_rng(0)
    x = rng.random((B, H, WD, K), dtype=np.float32)
    o = kernel(inputs=x)
    print("kernel out:", o.shape, o.dtype, "maxabs", np.abs(o).max())


# revision 33
# speedup vs baseline: 1.4693x; 1.4693x over previous
"""Trainium2 Bass kernel for nn_BotUpSaliency (B=2, H=W=512, K=12, 16 steps).

Math
----
The reference integrates, for 16 Euler steps (EPS=0.01):

    y'  = y + EPS*(-y + gx + conv(gx,W) + 1)
    x'  = x + EPS*(J0*gx + conv(gx,J) + inputs + i_norm - x - gy - gy@psi)
    gx  = clip(x - 1, 0, 1),  gy piecewise-linear,  out = mean_t gx_t, max over K

with x0 = 0.01, y0 = 1.  While gx == 0 (everywhere), the system collapses
exactly:
  * y stays exactly 1.0  (y + 0.01*(-1 + 0 + 0 + 1) == y), so gy == 0.21.
  * i_norm == 0.85 (conv of the all-zero s), conv(gx,*) == 0.
  * x_t = a_t * inputs + b_t elementwise, with scalar recurrences
        a_{t+1} = (1-EPS) a_t + EPS,           a_0 = 0
        b_{t+1} = (1-EPS) b_t + EPS*(0.85 - gy - colsum(psi)*gy),  b_0 = 0.01
  * gx_t = clip(a_t*inputs + b_t - 1, 0, 1) stays identically 0 as long as
        max_t (a_t * inputs.max() + b_t) < 1
    which requires inputs.max() >= ~6.66; the model's input domain is [0,1).

Hence out = (1/16) * sum_t clip(a_t*inputs + b_t - 1, 0, 1), and because each
term is nondecreasing in the input value, max over channels commutes with the
whole expression: it is evaluated at m = max_k inputs.

The device kernel computes exactly that: m = channel-max of the input slab
(reads all input bytes - the memory-bound part), then evaluates the sum of
affine-clip terms. Because the clip knots (1-b_t)/a_t decrease with t, for
m < (1-b_15)/a_15 ~= 7.075 the sum equals its t=16 term alone, so a single
relu-affine + min evaluates it exactly on the guard-certified domain.

A host-side guard verifies the collapse precondition (with wide margin) from
the actual inputs/psi and otherwise falls back to a full jax implementation
of the reference on CPU.

Sharding: pure data parallelism, 8 cores x 128 rows of the flattened
(2*512, 512, 12) input.
"""

import numpy as np

K = 12
STEPS = 16
EPS = 0.01
TX = 1.0
G1 = 0.21
J0 = 0.8
B, H, WD = 2, 512, 512
N_CORES = 8
ROWS = B * H                  # 1024 flattened rows
RPC = ROWS // N_CORES         # 128 rows per core == SBUF partitions
ROWW = WD * K                  # 6144 floats per row
# input is staged channel-major (host transpose): 12 planes of [rows, 512];
# DMA chunks of 2 planes pipeline across the two HWDGE rings
CHUNK_PLANES = (2, 2, 2, 2, 1, 1, 1, 1)
assert sum(CHUNK_PLANES) == K

_CACHE = {}


def _coeffs(colsum):
    """Scalar affine recurrence coefficients while gx == 0 (float64)."""
    gy = G1 * 1.0             # y stays exactly 1.0
    drive = 0.85 - gy - colsum * gy
    a, b = 0.0, 0.01
    A, Bc = [], []
    for _ in range(STEPS):
        a = (1.0 - EPS) * a + EPS
        b = (1.0 - EPS) * b + EPS * drive
        A.append(a)
        Bc.append(b)
    return np.array(A), np.array(Bc)


def _build_program(A, Bc):
    import concourse.bacc as bacc
    import concourse.mybir as mybir
    from concourse.tile import TileContext

    f32 = mybir.dt.float32
    bf16 = mybir.dt.bfloat16
    relu = mybir.ActivationFunctionType.Relu

    nc = bacc.Bacc("TRN2", target_bir_lowering=False, debug=False)
    x = nc.dram_tensor("x", [RPC, ROWW], bf16, kind="ExternalInput")
    out = nc.dram_tensor("out", [RPC, WD], f32, kind="ExternalOutput")

    with TileContext(nc) as tc:
        with (
            tc.tile_pool(name="inp", bufs=8) as inpool,
            tc.tile_pool(name="zs", bufs=8) as zpool,
            tc.tile_pool(name="one", bufs=1) as spool,
        ):
            # per-step activation biases (b_t - 1)/16 as [128,1] scalars; the
            # 1/16 out-scale is folded into scale/bias/min so acc IS the output
            btab = spool.tile([RPC, STEPS], f32, tag="btab")
            for s in range(STEPS):
                nc.gpsimd.memset(btab[:, s:s + 1], float((Bc[s] - 1.0) / STEPS))
            # warm the ACT Relu table during the DMA window (1.3us table load)
            zw = spool.tile([RPC, 1], f32, tag="zw")
            nc.scalar.activation(out=zw[:], in_=btab[:, 0:1], func=relu)
            # m = per-pixel channel max: pairwise bf16 TT maxes over channel
            # planes (334ns each in 2x mode vs ~1.5us strided reduce); chunk
            # DMAs alternate between the two HWDGE rings (SP + ACT)
            m = spool.tile([RPC, WD], bf16, tag="m")
            running = None
            col = 0
            for c, npl in enumerate(CHUNK_PLANES):
                t = inpool.tile([RPC, npl * WD], bf16, tag=f"in{npl}", name="t")
                dma_eng = nc.sync if c % 2 == 0 else nc.scalar
                dma_eng.dma_start(out=t[:], in_=x[:, col:col + npl * WD])
                col += npl * WD
                last = c == len(CHUNK_PLANES) - 1
                if npl == 2:
                    p = zpool.tile([RPC, WD], bf16, tag="pp", name="pp")
                    nc.vector.tensor_tensor(
                        out=p[:], in0=t[:, :WD], in1=t[:, WD:],
                        op=mybir.AluOpType.max)
                else:
                    p = t
                if running is None:
                    running = p
                else:
                    nxt = m if last else zpool.tile(
                        [RPC, WD], bf16, tag="rm", name="rm")
                    nc.vector.tensor_tensor(
                        out=nxt[:], in0=running[:], in1=p[:, :WD],
                        op=mybir.AluOpType.max)
                    running = nxt
            # acc = sum_t clip(a_t*m + b_t - 1, 0, 1); ACT does the affine+relu,
            # DVE runs the fused (min 1) + acc chain (STT is DVE-only; Pool's
            # tensor_scalar ucode is ~15x slower and contends on the SBUF port).
            # The clip knots (1-b_t)/a_t DECREASE with t, so for
            # m < (1-b_15)/a_15 ~= 7.075 only the t=16 term can be nonzero and
            # sum_t clip(a_t*m + b_t - 1, 0, 1) == clip(a_16*m + b_16 - 1, 0, 1)
            # exactly. The host guard certifies m < ~6.53, strictly inside.
            # One ACT (relu-affine) + one DVE min evaluate it, 1/16 folded in.
            z = spool.tile([RPC, WD], f32, tag="z")
            nc.scalar.activation(
                out=z[:], in_=m[:], func=relu,
                bias=btab[:, STEPS - 1:STEPS], scale=float(A[STEPS - 1] / STEPS),
            )
            acc = spool.tile([RPC, WD], f32, tag="acc")
            nc.vector.tensor_scalar_min(out=acc[:], in0=z[:], scalar1=1.0 / STEPS)
            nc.sync.dma_start(out=out[:], in_=acc[:])

    nc.compile()
    return nc


def _get_program(A, Bc):
    key = (tuple(np.round(A, 12)), tuple(np.round(Bc, 12)))
    if key not in _CACHE:
        _CACHE[key] = _build_program(A, Bc)
    return _CACHE[key]


def _run_on_device(inputs_np, A, Bc, trace=False):
    from concourse.bass_utils import run_bass_kernel_spmd

    nc = _get_program(A, Bc)
    import ml_dtypes
    flat = np.ascontiguousarray(
        inputs_np.reshape(ROWS, WD, K).transpose(0, 2, 1)
    ).astype(ml_dtypes.bfloat16).reshape(ROWS, ROWW)
    in_maps = [
        {"x": np.ascontiguousarray(flat[i * RPC:(i + 1) * RPC])}
        for i in range(N_CORES)
    ]
    res = run_bass_kernel_spmd(nc, in_maps, list(range(N_CORES)), trace=trace)
    out = np.concatenate([res.results[i]["out"] for i in range(N_CORES)], axis=0)
    return out.reshape(B, H, WD).astype(np.float32), res


def _reference_fallback(inputs, Wk, Jk, psi):
    """Full reference math in jax on CPU (only for out-of-domain inputs)."""
    import jax
    import jax.numpy as jnp

    cpu = jax.devices("cpu")[0]
    with jax.default_device(cpu):
        inputs = jnp.asarray(np.asarray(inputs), jnp.float32)
        Wk = jnp.asarray(np.asarray(Wk), jnp.float32)
        Jk = jnp.asarray(np.asarray(Jk), jnp.float32)
        psi = jnp.asarray(np.asarray(psi), jnp.float32)
        PAD = 7

        def _conv(xx, kk, padding):
            return jax.lax.conv_general_dilated(
                xx, kk, (1, 1), padding,
                dimension_numbers=("NHWC", "HWIO", "NHWC"))

        def _gx(xx):
            return jnp.clip(xx - TX, 0.0, 1.0)

        def _gy(yy):
            yc = jnp.maximum(yy, 0.0)
            return jnp.where(yc <= 1.2, G1 * yc, G1 * 1.2 + 2.5 * (yc - 1.2))

        psi_mat = psi[0, 0]
        box = jnp.ones((5, 5, 1, 1), inputs.dtype)
        x = jnp.full_like(inputs, 0.01)
        y = jnp.ones_like(inputs)
        gx = _gx(x)
        gy = _gy(y)
        out = jnp.zeros_like(inputs)
        for _ in range(STEPS):
            s = jnp.sum(gx, axis=3, keepdims=True)
            i_norm = 0.85 - 2.0 * (_conv(s, box, "SAME") / 25.0) ** 2
            gx_p = jnp.pad(gx, ((0, 0), (PAD, PAD), (PAD, PAD), (0, 0)),
                           mode="symmetric")
            inhib = _conv(gx_p, Wk, "VALID")
            excit = _conv(gx_p, Jk, "VALID")
            inhibs_psi = jnp.einsum("bhwi,io->bhwo", gy, psi_mat)
            y_new = y + EPS * (-y + gx + inhib + 1.0)
            x_inhib = x + gy + inhibs_psi
            x_excit = J0 * gx + excit + inputs + i_norm
            x_new = x + EPS * (x_excit - x_inhib)
            gx = _gx(x_new)
            gy = _gy(y_new)
            x, y = x_new, y_new
            out = out + gx
        out = out / STEPS
        return np.asarray(jnp.max(out, axis=3))


def kernel(inputs, W=None, J=None, psi=None, **_ignored):
    inputs_np = np.asarray(inputs, dtype=np.float32)
    assert inputs_np.shape == (B, H, WD, K), inputs_np.shape

    # Guard: the gx==0 collapse must hold for these inputs/psi.
    ok = True
    colsum = 3.0
    if psi is not None:
        cs = np.asarray(psi, dtype=np.float64)[0, 0].sum(axis=0)
        if np.max(np.abs(cs - cs[0])) < 1e-9:
            colsum = float(cs[0])
        else:
            ok = False
    if ok:
        A, Bc = _coeffs(colsum)
        # 1.004 factor covers bf16 round-up of the staged inputs (<= 2^-8 rel)
        mx = float(inputs_np.max()) * 1.004
        if np.max(A * mx + Bc) >= 0.98:
            ok = False
    if not ok:
        return _reference_fallback(inputs, W, J, psi).astype(np.float32)

    out, _ = _run_on_device(inputs_np, A, Bc)
    return out


if __name__ == "__main__":
    rng = np.random.default_rng(0)
    x = rng.random((B, H, WD, K), dtype=np.float32)
    o = kernel(inputs=x)
    print("kernel out:", o.shape, o.dtype, "maxabs", np.abs(o).max())


# revision 35
# speedup vs baseline: 1.5272x; 1.0394x over previous
"""Trainium2 Bass kernel for nn_BotUpSaliency (B=2, H=W=512, K=12, 16 steps).

Math
----
The reference integrates, for 16 Euler steps (EPS=0.01):

    y'  = y + EPS*(-y + gx + conv(gx,W) + 1)
    x'  = x + EPS*(J0*gx + conv(gx,J) + inputs + i_norm - x - gy - gy@psi)
    gx  = clip(x - 1, 0, 1),  gy piecewise-linear,  out = mean_t gx_t, max over K

with x0 = 0.01, y0 = 1.  While gx == 0 (everywhere), the system collapses
exactly:
  * y stays exactly 1.0  (y + 0.01*(-1 + 0 + 0 + 1) == y), so gy == 0.21.
  * i_norm == 0.85 (conv of the all-zero s), conv(gx,*) == 0.
  * x_t = a_t * inputs + b_t elementwise, with scalar recurrences
        a_{t+1} = (1-EPS) a_t + EPS,           a_0 = 0
        b_{t+1} = (1-EPS) b_t + EPS*(0.85 - gy - colsum(psi)*gy),  b_0 = 0.01
  * gx_t = clip(a_t*inputs + b_t - 1, 0, 1) stays identically 0 as long as
        max_t (a_t * inputs.max() + b_t) < 1
    which requires inputs.max() >= ~6.66; the model's input domain is [0,1).

Hence out = (1/16) * sum_t clip(a_t*inputs + b_t - 1, 0, 1), and because each
term is nondecreasing in the input value, max over channels commutes with the
whole expression: it is evaluated at m = max_k inputs.

The device kernel computes exactly that: m = channel-max of the input slab
(reads all input bytes - the memory-bound part), then evaluates the sum of
affine-clip terms. Because the clip knots (1-b_t)/a_t decrease with t, for
m < (1-b_15)/a_15 ~= 7.075 the sum equals its t=16 term alone, and that term
stays below 1/16 there, so a single relu-affine evaluates it exactly on the
guard-certified domain.

A host-side guard verifies the collapse precondition (with wide margin) from
the actual inputs/psi and otherwise falls back to a full jax implementation
of the reference on CPU.

Sharding: pure data parallelism, 8 cores x 128 rows of the flattened
(2*512, 512, 12) input.
"""

import numpy as np

K = 12
STEPS = 16
EPS = 0.01
TX = 1.0
G1 = 0.21
J0 = 0.8
B, H, WD = 2, 512, 512
N_CORES = 8
ROWS = B * H                  # 1024 flattened rows
RPC = ROWS // N_CORES         # 128 rows per core == SBUF partitions
ROWW = WD * K                  # 6144 floats per row
# input is staged channel-major (host transpose): 12 planes of [rows, 512];
# DMA chunks of 2 planes pipeline across the two HWDGE rings
CHUNK_PLANES = (2, 2, 2, 2, 1, 1, 1, 1)
assert sum(CHUNK_PLANES) == K

_CACHE = {}


def _coeffs(colsum):
    """Scalar affine recurrence coefficients while gx == 0 (float64)."""
    gy = G1 * 1.0             # y stays exactly 1.0
    drive = 0.85 - gy - colsum * gy
    a, b = 0.0, 0.01
    A, Bc = [], []
    for _ in range(STEPS):
        a = (1.0 - EPS) * a + EPS
        b = (1.0 - EPS) * b + EPS * drive
        A.append(a)
        Bc.append(b)
    return np.array(A), np.array(Bc)


def _build_program(A, Bc):
    import concourse.bacc as bacc
    import concourse.mybir as mybir
    from concourse.tile import TileContext

    f32 = mybir.dt.float32
    bf16 = mybir.dt.bfloat16
    relu = mybir.ActivationFunctionType.Relu

    nc = bacc.Bacc("TRN2", target_bir_lowering=False, debug=False)
    x = nc.dram_tensor("x", [RPC, ROWW], bf16, kind="ExternalInput")
    out = nc.dram_tensor("out", [RPC, WD], f32, kind="ExternalOutput")

    with TileContext(nc) as tc:
        with (
            tc.tile_pool(name="inp", bufs=8) as inpool,
            tc.tile_pool(name="zs", bufs=8) as zpool,
            tc.tile_pool(name="one", bufs=1) as spool,
        ):
            # per-step activation biases (b_t - 1)/16 as [128,1] scalars; the
            # 1/16 out-scale is folded into scale/bias/min so acc IS the output
            btab = spool.tile([RPC, STEPS], f32, tag="btab")
            for s in range(STEPS):
                nc.gpsimd.memset(btab[:, s:s + 1], float((Bc[s] - 1.0) / STEPS))
            # warm the ACT Relu table during the DMA window (1.3us table load)
            zw = spool.tile([RPC, 1], f32, tag="zw")
            nc.scalar.activation(out=zw[:], in_=btab[:, 0:1], func=relu)
            # m = per-pixel channel max: pairwise bf16 TT maxes over channel
            # planes (334ns each in 2x mode vs ~1.5us strided reduce); chunk
            # DMAs alternate between the two HWDGE rings (SP + ACT)
            m = spool.tile([RPC, WD], bf16, tag="m")
            running = None
            col = 0
            for c, npl in enumerate(CHUNK_PLANES):
                t = inpool.tile([RPC, npl * WD], bf16, tag=f"in{npl}", name="t")
                dma_eng = nc.sync if c % 2 == 0 else nc.scalar
                dma_eng.dma_start(out=t[:], in_=x[:, col:col + npl * WD])
                col += npl * WD
                last = c == len(CHUNK_PLANES) - 1
                if npl == 2:
                    p = zpool.tile([RPC, WD], bf16, tag="pp", name="pp")
                    nc.vector.tensor_tensor(
                        out=p[:], in0=t[:, :WD], in1=t[:, WD:],
                        op=mybir.AluOpType.max)
                else:
                    p = t
                if running is None:
                    running = p
                else:
                    nxt = m if last else zpool.tile(
                        [RPC, WD], bf16, tag="rm", name="rm")
                    nc.vector.tensor_tensor(
                        out=nxt[:], in0=running[:], in1=p[:, :WD],
                        op=mybir.AluOpType.max)
                    running = nxt
            # acc = sum_t clip(a_t*m + b_t - 1, 0, 1); ACT does the affine+relu,
            # DVE runs the fused (min 1) + acc chain (STT is DVE-only; Pool's
            # tensor_scalar ucode is ~15x slower and contends on the SBUF port).
            # The clip knots (1-b_t)/a_t DECREASE with t, so for
            # m < (1-b_15)/a_15 ~= 7.075 only the t=16 term can be nonzero and
            # sum_t clip(a_t*m + b_t - 1, 0, 1) == clip(a_16*m + b_16 - 1, 0, 1)
            # exactly. On that whole domain the term tops out at ~0.061 < 1,
            # so the upper clip can never bind either: one ACT relu-affine IS
            # the exact result (1/16 folded in). Host guard certifies m < 6.53.
            z = spool.tile([RPC, WD], f32, tag="z")
            nc.scalar.activation(
                out=z[:], in_=m[:], func=relu,
                bias=btab[:, STEPS - 1:STEPS], scale=float(A[STEPS - 1] / STEPS),
            )
            half = WD // 2
            nc.sync.dma_start(out=out[:, :half], in_=z[:, :half])
            nc.scalar.dma_start(out=out[:, half:], in_=z[:, half:])

    nc.compile()
    return nc


def _get_program(A, Bc):
    key = (tuple(np.round(A, 12)), tuple(np.round(Bc, 12)))
    if key not in _CACHE:
        _CACHE[key] = _build_program(A, Bc)
    return _CACHE[key]


def _run_on_device(inputs_np, A, Bc, trace=False):
    from concourse.bass_utils import run_bass_kernel_spmd

    nc = _get_program(A, Bc)
    import ml_dtypes
    flat = np.ascontiguousarray(
        inputs_np.reshape(ROWS, WD, K).transpose(0, 2, 1)
    ).astype(ml_dtypes.bfloat16).reshape(ROWS, ROWW)
    in_maps = [
        {"x": np.ascontiguousarray(flat[i * RPC:(i + 1) * RPC])}
        for i in range(N_CORES)
    ]
    res = run_bass_kernel_spmd(nc, in_maps, list(range(N_CORES)), trace=trace)
    out = np.concatenate([res.results[i]["out"] for i in range(N_CORES)], axis=0)
    return out.reshape(B, H, WD).astype(np.float32), res


def _reference_fallback(inputs, Wk, Jk, psi):
    """Full reference math in jax on CPU (only for out-of-domain inputs)."""
    import jax
    import jax.numpy as jnp

    cpu = jax.devices("cpu")[0]
    with jax.default_device(cpu):
        inputs = jnp.asarray(np.asarray(inputs), jnp.float32)
        Wk = jnp.asarray(np.asarray(Wk), jnp.float32)
        Jk = jnp.asarray(np.asarray(Jk), jnp.float32)
        psi = jnp.asarray(np.asarray(psi), jnp.float32)
        PAD = 7

        def _conv(xx, kk, padding):
            return jax.lax.conv_general_dilated(
                xx, kk, (1, 1), padding,
                dimension_numbers=("NHWC", "HWIO", "NHWC"))

        def _gx(xx):
            return jnp.clip(xx - TX, 0.0, 1.0)

        def _gy(yy):
            yc = jnp.maximum(yy, 0.0)
            return jnp.where(yc <= 1.2, G1 * yc, G1 * 1.2 + 2.5 * (yc - 1.2))

        psi_mat = psi[0, 0]
        box = jnp.ones((5, 5, 1, 1), inputs.dtype)
        x = jnp.full_like(inputs, 0.01)
        y = jnp.ones_like(inputs)
        gx = _gx(x)
        gy = _gy(y)
        out = jnp.zeros_like(inputs)
        for _ in range(STEPS):
            s = jnp.sum(gx, axis=3, keepdims=True)
            i_norm = 0.85 - 2.0 * (_conv(s, box, "SAME") / 25.0) ** 2
            gx_p = jnp.pad(gx, ((0, 0), (PAD, PAD), (PAD, PAD), (0, 0)),
                           mode="symmetric")
            inhib = _conv(gx_p, Wk, "VALID")
            excit = _conv(gx_p, Jk, "VALID")
            inhibs_psi = jnp.einsum("bhwi,io->bhwo", gy, psi_mat)
            y_new = y + EPS * (-y + gx + inhib + 1.0)
            x_inhib = x + gy + inhibs_psi
            x_excit = J0 * gx + excit + inputs + i_norm
            x_new = x + EPS * (x_excit - x_inhib)
            gx = _gx(x_new)
            gy = _gy(y_new)
            x, y = x_new, y_new
            out = out + gx
        out = out / STEPS
        return np.asarray(jnp.max(out, axis=3))


def kernel(inputs, W=None, J=None, psi=None, **_ignored):
    inputs_np = np.asarray(inputs, dtype=np.float32)
    assert inputs_np.shape == (B, H, WD, K), inputs_np.shape

    # Guard: the gx==0 collapse must hold for these inputs/psi.
    ok = True
    colsum = 3.0
    if psi is not None:
        cs = np.asarray(psi, dtype=np.float64)[0, 0].sum(axis=0)
        if np.max(np.abs(cs - cs[0])) < 1e-9:
            colsum = float(cs[0])
        else:
            ok = False
    if ok:
        A, Bc = _coeffs(colsum)
        # 1.004 factor covers bf16 round-up of the staged inputs (<= 2^-8 rel)
        mx = float(inputs_np.max()) * 1.004
        if np.max(A * mx + Bc) >= 0.98:
            ok = False
    if not ok:
        return _reference_fallback(inputs, W, J, psi).astype(np.float32)

    out, _ = _run_on_device(inputs_np, A, Bc)
    return out


if __name__ == "__main__":
    rng = np.random.default_rng(0)
    x = rng.random((B, H, WD, K), dtype=np.float32)
    o = kernel(inputs=x)
    print("kernel out:", o.shape, o.dtype, "maxabs", np.abs(o).max())
